# revision 38
# baseline (speedup 1.0000x reference)
"""DTNNStep (gnn message passing) on 8 Trainium2 NeuronCores.

Strategy (edge-parallel):
  * Edges (2M, sorted by membership_i) are sharded across 8 cores at atom
    boundaries: core c owns atoms [12500c, 12500(c+1)) and the edges whose
    destination (membership_i) falls in that range.
  * Within a core, edges split into 4 CONTIGUOUS quarters (substreams);
    quarter k is processed on partition block 32k..32k+30 of the packed
    pipeline below.
  * Host precomputes atom_hidden = af @ W_cf + b_cf once (f32), gathers
    a_h[mj] per edge (bf16), and builds the self-interaction correction
    base = af - tanh((b_df*a_h) @ W_fc) on the host.  The device only runs
    the edge pipeline:
      - psdh = [W_df; b_df]^T @ distT per quarter, pair-packed into
        [128, c] PSUM tiles (quarters 2p at partitions 0:64, 2p+1 at
        64:128 via the PE tile_position mechanism),
      - prod = psdh * gathered a_h  (gpsimd/Pool engine, bf16 out),
      - psoh = blockdiag(W_fc)^T @ prod  (one matmul covers 2 quarters),
      - tanh (ACT) -> f32, then an UNMASKED f32 running cumsum per
        partition (DVE tensor_tensor_scan with ones as data0).
    All DMA is HWDGE (loads on the sync engine queue, stores on the
    scalar engine queue) - no SWDGE descriptor overhead.
  * Host reads the cumsum at (host-known) segment-end columns and takes
    differences to get per-atom segment sums; adds base.
"""

import os
import sys

for _p in ("/opt/trn_rl_repo", "/root/.axon_site/_ro/trn_rl_repo"):
    if os.path.isdir(_p) and _p not in sys.path:
        sys.path.append(_p)

import numpy as np
from ml_dtypes import bfloat16
from contextlib import ExitStack

import concourse.bass as bass
import concourse.bacc as bacc
import concourse.mybir as mybir
import concourse.tile as tile
from concourse.bass_utils import run_bass_kernel_spmd

BF16 = mybir.dt.bfloat16
F32 = mybir.dt.float32
FP8 = mybir.dt.float8e3


class Cfg:
    def __init__(self, n_atoms=100000, n_emb=30, n_dist=100, n_hid=60,
                 n_cores=8, n_sub=4, c=512, jumbo=8192):
        self.n_atoms = n_atoms
        self.n_emb = n_emb
        self.n_dist = n_dist
        self.n_hid = n_hid
        self.n_cores = n_cores
        self.n_sub = n_sub
        self.c = c              # pipeline chunk columns (edges per quarter-chunk)
        self.jumbo = jumbo      # columns per bulk DMA (multiple of c)
        self.apc = n_atoms // n_cores
        assert jumbo % c == 0


DEFAULT_CFG = Cfg()


def build_program(cfg, cap):
    """Build + compile the (SPMD-identical) Bass program for one core."""
    c = cfg.c
    jb = cfg.jumbo
    nj = cap // jb
    tpj = jb // c
    assert cap % jb == 0
    H = cfg.n_hid

    nc = bacc.Bacc("TRN2", target_bir_lowering=False, debug=False,
                   num_devices=cfg.n_cores, num_swdge_queues=4)

    nd1 = cfg.n_dist + 1   # dist rows + ones row
    # distance mean-centered in fp8 e3m4 (4 mantissa bits): halves the
    # dominant DMA stream; the 0.5 mean is folded into the bias row of W_df
    distT = nc.dram_tensor("distT", [cfg.n_sub, nd1, cap], FP8,
                           kind="ExternalInput").ap()
    ahT = nc.dram_tensor("ahT", [2, 128, cap], FP8, kind="ExternalInput").ap()
    Wdf = nc.dram_tensor("Wdf", [nd1, 64], BF16, kind="ExternalInput").ap()
    Wfc2 = nc.dram_tensor("Wfc2", [128, 64], BF16, kind="ExternalInput").ap()
    scanout = nc.dram_tensor("scanout", [128, cap], F32,
                             kind="ExternalOutput").ap()

    distT_r = distT.rearrange("s r c -> r s c")
    ahT_r = ahT.rearrange("p r c -> r p c")

    with tile.TileContext(nc) as tc, ExitStack() as ctx:
        wpool = ctx.enter_context(tc.tile_pool(name="weights", bufs=1))
        wdf_sb = wpool.tile([nd1, 64], BF16)
        nc.sync.dma_start(wdf_sb[:], Wdf[:])
        wfc2_sb = wpool.tile([128, 64], BF16)
        nc.sync.dma_start(wfc2_sb[:], Wfc2[:])
        ones_sb = wpool.tile([128, c], F32)
        nc.vector.memset(ones_sb[:], 1.0)

        with tc.tile_pool(name="ep_d", bufs=2) as dpool, \
             tc.tile_pool(name="ep_a", bufs=2) as apool, \
             tc.tile_pool(name="ep_pr", bufs=6) as prpool, \
             tc.tile_pool(name="ep_th", bufs=4) as thpool, \
             tc.tile_pool(name="ep_st", bufs=2) as spool, \
             tc.tile_pool(name="ep_pd", bufs=4, space="PSUM") as psd, \
             tc.tile_pool(name="ep_po", bufs=3, space="PSUM") as pso:
            carry = None
            for j in range(nj):
                dj = dpool.tile([nd1, cfg.n_sub, jb], FP8, tag="dj")
                for k in range(cfg.n_sub):
                    # bulk dist stream via SWDGE (sprays all 16 DMA engines)
                    nc.gpsimd.dma_start(dj[:, k, :],
                                        distT_r[:, k, j * jb:(j + 1) * jb])
                aj = apool.tile([128, 2, jb], FP8, tag="aj")
                # gathered-atom-hidden reads balanced across the three read
                # paths: rows 0:118 of each pair on a HWDGE queue (sync /
                # scalar), the remainder rides the SWDGE stream
                nc.sync.dma_start(aj[0:118, 0, :],
                                  ahT_r[0:118, 0, j * jb:(j + 1) * jb])
                nc.scalar.dma_start(aj[0:118, 1, :],
                                    ahT_r[0:118, 1, j * jb:(j + 1) * jb])
                for p in range(2):
                    nc.gpsimd.dma_start(aj[118:128, p, :],
                                        ahT_r[118:128, p, j * jb:(j + 1) * jb])
                stg = spool.tile([128, jb], F32, tag="stg")
                for t in range(tpj):
                    c0 = t * c
                    pd = [psd.tile([128, c], F32, tag=f"pd{p}", bufs=2,
                                   name=f"pd{p}") for p in range(2)]
                    # dist-hidden matmuls, batched by PE array position so
                    # consecutive instructions share the stationary weights
                    for h in (0, 1):
                        for p in (0, 1):
                            k = 2 * p + h
                            nc.tensor.matmul(pd[p][64 * h:64 * h + 64],
                                             lhsT=wdf_sb[:],
                                             rhs=dj[:, k, c0:c0 + c],
                                             start=True, stop=True)
                    # prod = dist_hidden * gathered atom_hidden.  Pair 1 on
                    # DVE (reads PSUM directly); pair 0 via an ACT-engine
                    # PSUM->SBUF copy + Pool-engine multiply, spreading the
                    # elementwise work over three engines.
                    prods = []
                    dh0 = prpool.tile([128, c], BF16, tag="dh0", name="dh0")
                    nc.scalar.copy(dh0[:], pd[0][:])
                    prod0 = prpool.tile([128, c], BF16, tag="prod0",
                                        name="prod0")
                    nc.gpsimd.tensor_tensor(prod0[:], dh0[:],
                                            aj[:, 0, c0:c0 + c],
                                            op=mybir.AluOpType.mult)
                    prods.append(prod0)
                    prod1 = prpool.tile([128, c], BF16, tag="prod1",
                                        name="prod1")
                    nc.vector.tensor_tensor(prod1[:], pd[1][:],
                                            aj[:, 1, c0:c0 + c],
                                            op=mybir.AluOpType.mult)
                    prods.append(prod1)
                    # fc matmul: block-diag W_fc covers both quarters of a pair
                    po = pso.tile([128, c], F32, tag="po")
                    for p in (0, 1):
                        nc.tensor.matmul(po[64 * p:64 * p + 64],
                                         lhsT=wfc2_sb[:], rhs=prods[p][:],
                                         start=True, stop=True)
                    th = thpool.tile([128, c], F32, tag="th")
                    nc.scalar.activation(th[:], po[:],
                                         mybir.ActivationFunctionType.Tanh)
                    # f32 running cumsum along columns (per partition)
                    nc.vector.tensor_tensor_scan(
                        stg[:, c0:c0 + c], data0=ones_sb[:], data1=th[:],
                        initial=(0.0 if carry is None else carry),
                        op0=mybir.AluOpType.mult, op1=mybir.AluOpType.add)
                    carry = stg[:, c0 + c - 1:c0 + c]
                # store on the scalar-engine HWDGE queue so the sync-engine
                # queue stays a pure load-prefetch stream
                nc.scalar.dma_start(scanout[:, j * jb:(j + 1) * jb], stg[:])

    nc.compile()
    return nc


def host_prep(inputs, cfg):
    """Shard + lay out inputs for the 8 cores. Returns (in_maps, post_data, cap)."""
    af = np.asarray(inputs["atom_features"], dtype=np.float32)
    dist = np.asarray(inputs["distance"], dtype=np.float32)
    mi = np.asarray(inputs["distance_membership_i"]).astype(np.int64)
    mj = np.asarray(inputs["distance_membership_j"]).astype(np.int64)
    W_cf = np.asarray(inputs["W_cf"], dtype=np.float32)
    W_df = np.asarray(inputs["W_df"], dtype=np.float32)
    W_fc = np.asarray(inputs["W_fc"], dtype=np.float32)
    b_cf = np.asarray(inputs["b_cf"], dtype=np.float32)
    b_df = np.asarray(inputs["b_df"], dtype=np.float32)

    n_emb, n_dist, H = cfg.n_emb, cfg.n_dist, cfg.n_hid
    jb = cfg.jumbo

    from ml_dtypes import float8_e3m4
    # atom_hidden (f32) once on the host; fp8 e3m4 copy for the edge gather
    a_h = af @ W_cf + b_cf[None, :]
    a_h_bf = a_h.astype(float8_e3m4)
    # self-interaction correction + residual, host-side
    base = af - np.tanh((b_df[None, :] * a_h) @ W_fc)
    Wdf_dr = np.zeros((n_dist + 1, 64), np.float32)
    Wdf_dr[:n_dist, :H] = W_df
    # bias row absorbs the 0.5 mean removed from the fp8 distance stream
    Wdf_dr[n_dist, :H] = b_df + 0.5 * W_df.sum(axis=0)
    Wdf_dr = Wdf_dr.astype(bfloat16)
    Wfc2 = np.zeros((128, 64), np.float32)
    Wfc2[0:H, 0:n_emb] = W_fc
    Wfc2[64:64 + H, 32:32 + n_emb] = W_fc
    Wfc2 = Wfc2.astype(bfloat16)

    bounds = np.searchsorted(mi, np.arange(0, cfg.n_atoms + 1, cfg.apc))
    caps = []
    for cid in range(cfg.n_cores):
        E = bounds[cid + 1] - bounds[cid]
        q = -(-E // cfg.n_sub)
        caps.append(-(-q // jb) * jb)
    cap = max(max(caps), jb)

    in_maps = []
    post_data = []
    for cid in range(cfg.n_cores):
        A0 = cid * cfg.apc
        e0, e1 = bounds[cid], bounds[cid + 1]
        E = e1 - e0
        q = -(-E // cfg.n_sub)
        distT = np.zeros((cfg.n_sub, n_dist + 1, cap), float8_e3m4)
        ahT = np.zeros((2, 128, cap), float8_e3m4)
        ends_k = []
        for k in range(cfg.n_sub):
            s = e0 + k * q
            e = min(e0 + (k + 1) * q, e1)
            n = e - s
            if n > 0:
                distT[k, :n_dist, :n] = (dist[s:e].T - 0.5).astype(float8_e3m4)
                distT[k, n_dist, :n] = float8_e3m4(1.0)
                ahT[k // 2, 64 * (k % 2):64 * (k % 2) + H, :n] = a_h_bf[mj[s:e]].T
                ids = mi[s:e]
                endpos = np.nonzero(np.r_[ids[1:] != ids[:-1], True])[0]
                ends_k.append((endpos.astype(np.int64),
                               (ids[endpos] - A0).astype(np.int64)))
            else:
                ends_k.append((np.zeros(0, np.int64), np.zeros(0, np.int64)))
        in_maps.append(dict(distT=distT, ahT=ahT, Wdf=Wdf_dr, Wfc2=Wfc2))
        post_data.append((ends_k, base[A0:A0 + cfg.apc]))
    return in_maps, post_data, cap


def host_post(results, post_data, cfg):
    out = np.empty((cfg.n_atoms, cfg.n_emb), np.float32)
    for cid in range(cfg.n_cores):
        ends_k, base_slice = post_data[cid]
        agg = base_slice.copy()
        sc = np.asarray(results[cid]["scanout"])  # f32 [128, cap]
        for k in range(cfg.n_sub):
            endpos, atoms = ends_k[k]
            if len(endpos):
                cum = sc[32 * k:32 * k + cfg.n_emb][:, endpos]
                seg = cum.copy()
                seg[:, 1:] -= cum[:, :-1]
                np.add.at(agg, atoms, seg.T)
        out[cid * cfg.apc:(cid + 1) * cfg.apc] = agg
    return out


_CACHE = {}


def kernel(**inputs):
    cfg = DEFAULT_CFG
    in_maps, post_data, cap = host_prep(inputs, cfg)
    if cap not in _CACHE:
        _CACHE[cap] = build_program(cfg, cap)
    nc = _CACHE[cap]
    res = run_bass_kernel_spmd(nc, in_maps, core_ids=list(range(cfg.n_cores)))
    return host_post(res.results, post_data, cfg)


# revision 45
# speedup vs baseline: 1.4410x; 1.4410x over previous
"""DTNNStep (gnn message passing) on 8 Trainium2 NeuronCores.

Strategy (edge-parallel):
  * Edges (2M, sorted by membership_i) are sharded across 8 cores at atom
    boundaries: core c owns atoms [12500c, 12500(c+1)) and the edges whose
    destination (membership_i) falls in that range.
  * Within a core, edges split into 4 CONTIGUOUS quarters (substreams);
    quarter k is processed on partition block 32k..32k+30 of the packed
    pipeline below.
  * Host precomputes atom_hidden = af @ W_cf + b_cf once (f32), gathers
    a_h[mj] per edge (bf16), and builds the self-interaction correction
    base = af - tanh((b_df*a_h) @ W_fc) on the host.  The device only runs
    the edge pipeline:
      - psdh = [W_df; b_df]^T @ distT per quarter, pair-packed into
        [128, c] PSUM tiles (quarters 2p at partitions 0:64, 2p+1 at
        64:128 via the PE tile_position mechanism),
      - prod = psdh * gathered a_h  (gpsimd/Pool engine, bf16 out),
      - psoh = blockdiag(W_fc)^T @ prod  (one matmul covers 2 quarters),
      - tanh (ACT) -> f32, then an UNMASKED f32 running cumsum per
        partition (DVE tensor_tensor_scan with ones as data0).
    All DMA is HWDGE (loads on the sync engine queue, stores on the
    scalar engine queue) - no SWDGE descriptor overhead.
  * Host reads the cumsum at (host-known) segment-end columns and takes
    differences to get per-atom segment sums; adds base.
"""

import os
import sys

for _p in ("/opt/trn_rl_repo", "/root/.axon_site/_ro/trn_rl_repo"):
    if os.path.isdir(_p) and _p not in sys.path:
        sys.path.append(_p)

import numpy as np
from ml_dtypes import bfloat16
from contextlib import ExitStack

import concourse.bass as bass
import concourse.bacc as bacc
import concourse.mybir as mybir
import concourse.tile as tile
from concourse.bass_utils import run_bass_kernel_spmd

BF16 = mybir.dt.bfloat16
F32 = mybir.dt.float32
FP8 = mybir.dt.float8e3


class Cfg:
    def __init__(self, n_atoms=100000, n_emb=30, n_dist=100, n_hid=60,
                 n_cores=8, n_sub=4, c=512, jumbo=4096):
        self.n_atoms = n_atoms
        self.n_emb = n_emb
        self.n_dist = n_dist
        self.n_hid = n_hid
        self.n_cores = n_cores
        self.n_sub = n_sub
        self.c = c              # pipeline chunk columns (edges per quarter-chunk)
        self.jumbo = jumbo      # columns per bulk DMA (multiple of c)
        self.apc = n_atoms // n_cores
        assert jumbo % c == 0


DEFAULT_CFG = Cfg()


def build_program(cfg, cap):
    """Build + compile the (SPMD-identical) Bass program for one core."""
    c = cfg.c
    jb = cfg.jumbo
    nj = cap // jb
    tpj = jb // c
    assert cap % jb == 0
    H = cfg.n_hid

    nc = bacc.Bacc("TRN2", target_bir_lowering=False, debug=False,
                   num_devices=cfg.n_cores, num_swdge_queues=4)

    nd1 = cfg.n_dist + 1   # dist rows + ones row
    # distance mean-centered in fp8 e3m4 (4 mantissa bits): halves the
    # dominant DMA stream; the 0.5 mean is folded into the bias row of W_df
    distT = nc.dram_tensor("distT", [cfg.n_sub, nd1, cap], FP8,
                           kind="ExternalInput").ap()
    ahT = nc.dram_tensor("ahT", [2, 128, cap], BF16, kind="ExternalInput").ap()
    Wdf = nc.dram_tensor("Wdf", [nd1, 64], BF16, kind="ExternalInput").ap()
    Wfc2 = nc.dram_tensor("Wfc2", [128, 64], BF16, kind="ExternalInput").ap()
    scanout = nc.dram_tensor("scanout", [128, cap], F32,
                             kind="ExternalOutput").ap()

    distT_r = distT.rearrange("s r c -> r s c")
    ahT_r = ahT.rearrange("p r c -> r p c")

    with tile.TileContext(nc) as tc, ExitStack() as ctx:
        wpool = ctx.enter_context(tc.tile_pool(name="weights", bufs=1))
        wdf_sb = wpool.tile([nd1, 64], BF16)
        nc.sync.dma_start(wdf_sb[:], Wdf[:])
        wfc2_sb = wpool.tile([128, 64], BF16)
        nc.sync.dma_start(wfc2_sb[:], Wfc2[:])
        ones_sb = wpool.tile([128, c], F32)
        nc.vector.memset(ones_sb[:], 1.0)

        with tc.tile_pool(name="ep_d", bufs=3) as dpool, \
             tc.tile_pool(name="ep_a", bufs=3) as apool, \
             tc.tile_pool(name="ep_pr", bufs=6) as prpool, \
             tc.tile_pool(name="ep_th", bufs=4) as thpool, \
             tc.tile_pool(name="ep_st", bufs=2) as spool, \
             tc.tile_pool(name="ep_pd", bufs=4, space="PSUM") as psd, \
             tc.tile_pool(name="ep_po", bufs=3, space="PSUM") as pso:
            carry = None
            for j in range(nj):
                dj = dpool.tile([nd1, cfg.n_sub, jb], FP8, tag="dj")
                for k in range(cfg.n_sub):
                    # bulk dist stream via SWDGE (sprays all 16 DMA engines)
                    nc.gpsimd.dma_start(dj[:, k, :],
                                        distT_r[:, k, j * jb:(j + 1) * jb])
                aj = apool.tile([128, 2, jb], BF16, tag="aj")
                # gathered-atom-hidden reads balanced across the three read
                # paths: rows 0:91 of each pair on a HWDGE queue (sync /
                # scalar), the remainder rides the SWDGE stream
                nc.sync.dma_start(aj[0:91, 0, :],
                                  ahT_r[0:91, 0, j * jb:(j + 1) * jb])
                nc.scalar.dma_start(aj[0:91, 1, :],
                                    ahT_r[0:91, 1, j * jb:(j + 1) * jb])
                for p in range(2):
                    nc.gpsimd.dma_start(aj[91:128, p, :],
                                        ahT_r[91:128, p, j * jb:(j + 1) * jb])
                stg = spool.tile([128, jb], F32, tag="stg")
                for t in range(tpj):
                    c0 = t * c
                    pd = [psd.tile([128, c], F32, tag=f"pd{p}", bufs=2,
                                   name=f"pd{p}") for p in range(2)]
                    # dist-hidden matmuls, batched by PE array position so
                    # consecutive instructions share the stationary weights
                    for h in (0, 1):
                        for p in (0, 1):
                            k = 2 * p + h
                            nc.tensor.matmul(pd[p][64 * h:64 * h + 64],
                                             lhsT=wdf_sb[:],
                                             rhs=dj[:, k, c0:c0 + c],
                                             start=True, stop=True)
                    # prod = dist_hidden * gathered atom_hidden (DVE; gpsimd
                    # cannot read PSUM on TRN2)
                    prods = []
                    for p in (0, 1):
                        prod = prpool.tile([128, c], BF16, tag=f"prod{p}",
                                           name=f"prod{p}")
                        nc.vector.tensor_tensor(prod[:], pd[p][:],
                                                aj[:, p, c0:c0 + c],
                                                op=mybir.AluOpType.mult)
                        prods.append(prod)
                    # fc matmul: block-diag W_fc covers both quarters of a pair
                    po = pso.tile([128, c], F32, tag="po")
                    for p in (0, 1):
                        nc.tensor.matmul(po[64 * p:64 * p + 64],
                                         lhsT=wfc2_sb[:], rhs=prods[p][:],
                                         start=True, stop=True)
                    th = thpool.tile([128, c], F32, tag="th")
                    nc.scalar.activation(th[:], po[:],
                                         mybir.ActivationFunctionType.Tanh)
                    # f32 running cumsum along columns (per partition)
                    nc.vector.tensor_tensor_scan(
                        stg[:, c0:c0 + c], data0=ones_sb[:], data1=th[:],
                        initial=(0.0 if carry is None else carry),
                        op0=mybir.AluOpType.mult, op1=mybir.AluOpType.add)
                    carry = stg[:, c0 + c - 1:c0 + c]
                # store on the scalar-engine HWDGE queue so the sync-engine
                # queue stays a pure load-prefetch stream
                nc.scalar.dma_start(scanout[:, j * jb:(j + 1) * jb], stg[:])

    nc.compile()
    return nc


def host_prep(inputs, cfg):
    """Shard + lay out inputs for the 8 cores. Returns (in_maps, post_data, cap)."""
    af = np.asarray(inputs["atom_features"], dtype=np.float32)
    dist = np.asarray(inputs["distance"], dtype=np.float32)
    mi = np.asarray(inputs["distance_membership_i"]).astype(np.int64)
    mj = np.asarray(inputs["distance_membership_j"]).astype(np.int64)
    W_cf = np.asarray(inputs["W_cf"], dtype=np.float32)
    W_df = np.asarray(inputs["W_df"], dtype=np.float32)
    W_fc = np.asarray(inputs["W_fc"], dtype=np.float32)
    b_cf = np.asarray(inputs["b_cf"], dtype=np.float32)
    b_df = np.asarray(inputs["b_df"], dtype=np.float32)

    n_emb, n_dist, H = cfg.n_emb, cfg.n_dist, cfg.n_hid
    jb = cfg.jumbo

    from ml_dtypes import float8_e3m4
    # atom_hidden (f32) once on the host; bf16 copy for the edge gather
    a_h = af @ W_cf + b_cf[None, :]
    a_h_bf = a_h.astype(bfloat16)
    # self-interaction correction + residual, host-side
    base = af - np.tanh((b_df[None, :] * a_h) @ W_fc)
    Wdf_dr = np.zeros((n_dist + 1, 64), np.float32)
    Wdf_dr[:n_dist, :H] = W_df
    # bias row absorbs the 0.5 mean removed from the fp8 distance stream
    Wdf_dr[n_dist, :H] = b_df + 0.5 * W_df.sum(axis=0)
    Wdf_dr = Wdf_dr.astype(bfloat16)
    Wfc2 = np.zeros((128, 64), np.float32)
    Wfc2[0:H, 0:n_emb] = W_fc
    Wfc2[64:64 + H, 32:32 + n_emb] = W_fc
    Wfc2 = Wfc2.astype(bfloat16)

    bounds = np.searchsorted(mi, np.arange(0, cfg.n_atoms + 1, cfg.apc))
    caps = []
    for cid in range(cfg.n_cores):
        E = bounds[cid + 1] - bounds[cid]
        q = -(-E // cfg.n_sub)
        caps.append(-(-q // jb) * jb)
    cap = max(max(caps), jb)

    in_maps = []
    post_data = []
    for cid in range(cfg.n_cores):
        A0 = cid * cfg.apc
        e0, e1 = bounds[cid], bounds[cid + 1]
        E = e1 - e0
        q = -(-E // cfg.n_sub)
        distT = np.zeros((cfg.n_sub, n_dist + 1, cap), float8_e3m4)
        ahT = np.zeros((2, 128, cap), bfloat16)
        ends_k = []
        for k in range(cfg.n_sub):
            s = e0 + k * q
            e = min(e0 + (k + 1) * q, e1)
            n = e - s
            if n > 0:
                distT[k, :n_dist, :n] = (dist[s:e].T - 0.5).astype(float8_e3m4)
                distT[k, n_dist, :n] = float8_e3m4(1.0)
                ahT[k // 2, 64 * (k % 2):64 * (k % 2) + H, :n] = a_h_bf[mj[s:e]].T
                ids = mi[s:e]
                endpos = np.nonzero(np.r_[ids[1:] != ids[:-1], True])[0]
                ends_k.append((endpos.astype(np.int64),
                               (ids[endpos] - A0).astype(np.int64)))
            else:
                ends_k.append((np.zeros(0, np.int64), np.zeros(0, np.int64)))
        in_maps.append(dict(distT=distT, ahT=ahT, Wdf=Wdf_dr, Wfc2=Wfc2))
        post_data.append((ends_k, base[A0:A0 + cfg.apc]))
    return in_maps, post_data, cap


def host_post(results, post_data, cfg):
    out = np.empty((cfg.n_atoms, cfg.n_emb), np.float32)
    for cid in range(cfg.n_cores):
        ends_k, base_slice = post_data[cid]
        agg = base_slice.copy()
        sc = np.asarray(results[cid]["scanout"])  # f32 [128, cap]
        for k in range(cfg.n_sub):
            endpos, atoms = ends_k[k]
            if len(endpos):
                cum = sc[32 * k:32 * k + cfg.n_emb][:, endpos]
                seg = cum.copy()
                seg[:, 1:] -= cum[:, :-1]
                np.add.at(agg, atoms, seg.T)
        out[cid * cfg.apc:(cid + 1) * cfg.apc] = agg
    return out


_CACHE = {}


def kernel(**inputs):
    cfg = DEFAULT_CFG
    in_maps, post_data, cap = host_prep(inputs, cfg)
    if cap not in _CACHE:
        _CACHE[cap] = build_program(cfg, cap)
    nc = _CACHE[cap]
    res = run_bass_kernel_spmd(nc, in_maps, core_ids=list(range(cfg.n_cores)))
    return host_post(res.results, post_data, cfg)
